# revision 1
# baseline (speedup 1.0000x reference)
"""Trainium2 Bass kernel for nn_MultiHeadAttention (dense transformer block:
qkv proj + RoPE + causal SDPA + out proj), tensor-parallel over (batch, heads)
across 8 NeuronCores.

Sharding: 2 batches x 16 heads = 32 (b,h) pairs; core c handles batch c//4,
heads 4*(c%4)..4*(c%4)+3. Each core computes qkv for its 4 heads (from the
full x of its batch), RoPE, causal attention, and a PARTIAL output
projection (its heads' rows of Wproj); the host sums the 4 partials per
batch. All matmuls run in bf16 (fp32 PSUM accumulation).

Layout notes:
- x is passed pre-transposed per batch (xT [D, S]) so the contraction dim
  (model dim) lands on SBUF partitions with no on-device transpose.
- q/k head dims are permuted host-side into a 16-interleaved (even,odd)
  order so RoPE's pair swap is a quadrant-local DVE stream_shuffle.
  Attention scores are invariant to this (q and k permuted identically).
- Scores are computed transposed (S^T [kv, q]) so softmax's denominator
  comes from a ones-matmul (column sums) and P^T feeds the O = V^T @ P^T
  matmul directly. exp() runs without max-subtraction: |scores| < ~10 for
  this input distribution, safe in fp32.
"""
import sys

sys.path.insert(0, "/opt/trn_rl_repo")

import numpy as np
import ml_dtypes

import concourse.bass as bass
import concourse.mybir as mybir
import concourse.tile as tile

P = 128
B, S, D = 2, 2048, 2048
NH, HD = 16, 128
NH_CORE = 4  # heads per core
HCOLS = NH_CORE * HD  # 512
KT = D // P  # 16 k-tiles
TT = S // P  # 16 token tiles
QC = 512  # q-chunk width
NQC = S // QC  # 4
ROPE_THETA = 10000.0
SCALE = HD**-0.5
NEG = -30000.0

F32 = mybir.dt.float32
BF16 = mybir.dt.bfloat16

_SWAP16 = [(i + 16) % 32 for i in range(32)]


# ---------------------------------------------------------------------------
# host-side constant tables
# ---------------------------------------------------------------------------
def _dim_perm():
    """Permutation p -> original head-dim index, 16-interleaved even/odd."""
    perm = np.zeros(HD, dtype=np.int64)
    for p in range(HD):
        qd, sl = p // 32, p % 32
        i = 16 * qd + (sl % 16)
        perm[p] = 2 * i if sl < 16 else 2 * i + 1
    return perm


def _rope_tables():
    """ctab[p,t], stab[p,t] (sign-baked) for the permuted head-dim layout."""
    perm = _dim_perm()
    inv_freq = 1.0 / (ROPE_THETA ** (np.arange(0, HD, 2, dtype=np.float64) / HD))
    t = np.arange(S, dtype=np.float64)
    ctab = np.zeros((HD, S), dtype=np.float64)
    stab = np.zeros((HD, S), dtype=np.float64)
    for p in range(HD):
        qd, sl = p // 32, p % 32
        i = 16 * qd + (sl % 16)
        ang = t * inv_freq[i]
        ctab[p] = np.cos(ang)
        stab[p] = -np.sin(ang) if sl < 16 else np.sin(ang)
    return ctab.astype(np.float32), stab.astype(np.float32)


def _tri_mask():
    """[P, P] f32: 0 where kv(row) <= q(col) else NEG."""
    b = np.arange(P)[:, None]
    a = np.arange(P)[None, :]
    return np.where(b <= a, 0.0, NEG).astype(np.float32)


# ---------------------------------------------------------------------------
# device kernel
# ---------------------------------------------------------------------------
def _build_nc():
    nc = bass.Bass()

    xT = nc.declare_dram_parameter("xT", [D, S], BF16, isOutput=False)
    Wq = nc.declare_dram_parameter("Wq", [D, HCOLS], BF16, isOutput=False)
    Wk = nc.declare_dram_parameter("Wk", [D, HCOLS], BF16, isOutput=False)
    Wv = nc.declare_dram_parameter("Wv", [D, HCOLS], BF16, isOutput=False)
    Wp = nc.declare_dram_parameter("Wp", [HCOLS, D], BF16, isOutput=False)
    out = nc.declare_dram_parameter("out", [S, D], F32, isOutput=True)

    # sqrt(SCALE) on both q and k tables => scores scaled by SCALE
    ctab_np, stab_np = _rope_tables()
    rt = np.sqrt(SCALE).astype(np.float32)
    cq_d = nc.inline_tensor((ctab_np * rt).astype(ml_dtypes.bfloat16), "cq")
    sq_d = nc.inline_tensor((stab_np * rt).astype(ml_dtypes.bfloat16), "sq")
    mask_d = nc.inline_tensor(_tri_mask(), "trimask")

    xT_t = xT[:].rearrange("(ko p) t -> p ko t", p=P)
    Wq_t = Wq[:].rearrange("(ko p) m -> p ko m", p=P)
    Wk_t = Wk[:].rearrange("(ko p) m -> p ko m", p=P)
    Wv_t = Wv[:].rearrange("(ko p) m -> p ko m", p=P)
    Wp_t = Wp[:].rearrange("(ho p) n -> p ho n", p=P)
    out_t = out[:].rearrange("(to p) n -> p to n", p=P)

    with tile.TileContext(nc) as tc:
        with (
            tc.tile_pool(name="persist", bufs=1) as pp,
            tc.tile_pool(name="work", bufs=2) as wk,
        ):
            # persistent tiles
            cq = pp.tile([P, S], BF16)
            sq = pp.tile([P, S], BF16)
            nc.sync.dma_start(cq, cq_d[:])
            nc.sync.dma_start(sq, sq_d[:])
            trimask = pp.tile([P, P], F32)
            nc.sync.dma_start(trimask, mask_d[:])
            ones_sb = pp.tile([P, P], BF16)
            nc.vector.memset(ones_sb, 1.0)

            Qt = pp.tile([P, NH_CORE, S], BF16)
            Kt = pp.tile([P, NH_CORE, S], BF16)
            Vt = pp.tile([P, TT, HCOLS], BF16)
            Yt = pp.tile([P, NH_CORE, S], BF16)

            # ---------------- phase 1: qkv projection + RoPE ---------------
            with (
                tc.tile_pool(name="mm1", bufs=1) as mm1p,
                tc.tile_pool(name="ps_mm1", bufs=8, space="PSUM") as psA,
            ):
                xT_sb = mm1p.tile([P, KT, S], BF16)
                Wq_sb = mm1p.tile([P, KT, HCOLS], BF16)
                Wk_sb = mm1p.tile([P, KT, HCOLS], BF16)
                Wv_sb = mm1p.tile([P, KT, HCOLS], BF16)
                for ki in range(KT):
                    nc.sync.dma_start(xT_sb[:, ki], xT_t[:, ki])
                    nc.gpsimd.dma_start(Wq_sb[:, ki], Wq_t[:, ki])
                    nc.gpsimd.dma_start(Wk_sb[:, ki], Wk_t[:, ki])
                    nc.gpsimd.dma_start(Wv_sb[:, ki], Wv_t[:, ki])

                # q and k projections with fused RoPE
                for W_sb, O_t, ctb, stb in (
                    (Wq_sb, Qt, cq, sq),
                    (Wk_sb, Kt, cq, sq),
                ):
                    for h in range(NH_CORE):
                        for tcx in range(NQC):
                            ps = psA.tile([P, QC], F32, tag="ps")
                            for ki in range(KT):
                                nc.tensor.matmul(
                                    ps,
                                    W_sb[:, ki, h * HD : (h + 1) * HD],
                                    xT_sb[:, ki, tcx * QC : (tcx + 1) * QC],
                                    start=(ki == 0),
                                    stop=(ki == KT - 1),
                                )
                            csl = ctb[:, tcx * QC : (tcx + 1) * QC]
                            ssl = stb[:, tcx * QC : (tcx + 1) * QC]
                            pc = wk.tile([P, QC], BF16, tag="pc")
                            nc.scalar.activation(
                                pc, ps, mybir.ActivationFunctionType.Copy
                            )
                            xsw = wk.tile([P, QC], BF16, tag="xsw")
                            nc.vector.stream_shuffle(xsw, pc, _SWAP16)
                            m1 = wk.tile([P, QC], BF16, tag="m1")
                            nc.vector.tensor_mul(m1, pc, csl)
                            m2 = wk.tile([P, QC], BF16, tag="m2")
                            nc.vector.tensor_mul(m2, xsw, ssl)
                            nc.vector.tensor_add(
                                O_t[:, h, tcx * QC : (tcx + 1) * QC], m1, m2
                            )

                # v projection (token-major)
                for tt in range(TT):
                    ps = psA.tile([P, HCOLS], F32, tag="ps")
                    for ki in range(KT):
                        nc.tensor.matmul(
                            ps,
                            xT_sb[:, ki, tt * P : (tt + 1) * P],
                            Wv_sb[:, ki],
                            start=(ki == 0),
                            stop=(ki == KT - 1),
                        )
                    nc.scalar.activation(
                        Vt[:, tt], ps, mybir.ActivationFunctionType.Copy
                    )

            # ---------------- phase 2: attention + out projection ----------
            with tc.tile_pool(name="attn", bufs=1) as atp:
                Wp_sb = atp.tile([P, NH_CORE, D], BF16)
                for ho in range(NH_CORE):
                    nc.sync.dma_start(Wp_sb[:, ho], Wp_t[:, ho])

                with (
                    tc.tile_pool(name="pt", bufs=3) as ptp,
                    tc.tile_pool(name="ps_o", bufs=2, space="PSUM") as psO,
                    tc.tile_pool(name="ps_s", bufs=5, space="PSUM") as psS,
                    tc.tile_pool(name="ps_l", bufs=1, space="PSUM") as psL,
                ):
                  for h in range(NH_CORE):
                    for qc in range(NQC):
                        o_ps = psO.tile([P, QC], F32, tag="ops")
                        l_ps = psL.tile([P, QC], F32, tag="lps")
                        njb = 4 * qc + 4
                        for jb in range(njb):
                            d = jb - 4 * qc  # diag offset if >= 0
                            off = 128 * d if d > 0 else 0
                            s_ps = psS.tile([P, QC], F32, tag="sps")
                            nc.tensor.matmul(
                                s_ps[:, off:],
                                Kt[:, h, jb * P : (jb + 1) * P],
                                Qt[:, h, qc * QC + off : (qc + 1) * QC],
                                start=True,
                                stop=True,
                            )
                            if d >= 0:
                                nc.vector.tensor_add(
                                    s_ps[:, off : off + P],
                                    s_ps[:, off : off + P],
                                    trimask,
                                )
                            pt = ptp.tile([P, QC], BF16, tag="pt")
                            nc.scalar.activation(
                                pt[:, off:],
                                s_ps[:, off:],
                                mybir.ActivationFunctionType.Exp,
                            )
                            nc.tensor.matmul(
                                o_ps[:, off:],
                                Vt[:, jb, h * HD : (h + 1) * HD],
                                pt[:, off:],
                                start=(jb == 0),
                                stop=(jb == njb - 1),
                            )
                            nc.tensor.matmul(
                                l_ps[:, off:],
                                ones_sb,
                                pt[:, off:],
                                start=(jb == 0),
                                stop=(jb == njb - 1),
                            )
                        rinv = wk.tile([P, QC], F32, tag="rinv")
                        nc.vector.reciprocal(rinv, l_ps)
                        nc.vector.tensor_mul(
                            Yt[:, h, qc * QC : (qc + 1) * QC], o_ps, rinv
                        )

                # out projection (partial; host sums across 4 cores/batch)
                with (
                    tc.tile_pool(name="outp", bufs=3) as outp,
                    tc.tile_pool(name="ps_p", bufs=6, space="PSUM") as psP,
                ):
                    for tt in range(TT):
                        ob = outp.tile([P, D], F32, tag="ob")
                        for ncx in range(D // QC):
                            ps = psP.tile([P, QC], F32, tag="psp")
                            for ho in range(NH_CORE):
                                nc.tensor.matmul(
                                    ps,
                                    Yt[:, ho, tt * P : (tt + 1) * P],
                                    Wp_sb[:, ho, ncx * QC : (ncx + 1) * QC],
                                    start=(ho == 0),
                                    stop=(ho == NH_CORE - 1),
                                )
                            nc.any.tensor_copy(
                                ob[:, ncx * QC : (ncx + 1) * QC], ps
                            )
                        nc.sync.dma_start(out_t[:, tt], ob)
    return nc


# ---------------------------------------------------------------------------
# legalization: this walrus build supports only ONE sync wait per instruction
# ---------------------------------------------------------------------------
_ENGINE_SEM_PREFIX = {
    "PE": "PE_",
    "DVE": "DVE_",
    "ACT": "ACT_",
    "Pool": "POOL_",
    "SP": "SP_",
}
_wf_counter = [0]


def _legalize(nc, max_waits=1):
    for f in nc.m.functions:
        for bb in f.blocks:
            new_insts = []
            for inst in bb.instructions:
                si = getattr(inst, "sync_info", None)
                eng = getattr(inst, "engine", None)
                if si is None or not si.on_wait or eng is None:
                    new_insts.append(inst)
                    continue
                waits = list(si.on_wait)
                pref = _ENGINE_SEM_PREFIX.get(eng.name)
                if pref is not None:
                    waits = [
                        w
                        for w in waits
                        if not (
                            w.sync_type == "semaphore"
                            and w.ant_name.startswith(pref)
                        )
                    ]
                if len(waits) > max_waits:
                    for w in waits[:-max_waits]:
                        _wf_counter[0] += 1
                        nop = mybir.InstNoOp(
                            name=f"I-waitfix-{_wf_counter[0]}", ins=[], outs=[]
                        )
                        nop.engine = eng
                        nop.sync_info = mybir.SyncInfo(on_wait=[w], on_update=[])
                        new_insts.append(nop)
                    waits = waits[-max_waits:]
                if len(waits) != len(si.on_wait):
                    inst.sync_info = mybir.SyncInfo(
                        on_wait=waits, on_update=list(si.on_update)
                    )
                new_insts.append(inst)
            bb.instructions[:] = new_insts


# ---------------------------------------------------------------------------
# SPMD runner (mirrors concourse.bass2jax.run_bass_via_pjrt, kept resident)
# ---------------------------------------------------------------------------
class _Runner:
    def __init__(self, nc, n_cores=8):
        import jax
        from jax.sharding import Mesh, PartitionSpec
        from jax.experimental.shard_map import shard_map
        from concourse import bass2jax
        from concourse.bass2jax import _bass_exec_p, install_neuronx_cc_hook

        install_neuronx_cc_hook()
        self.jax = jax
        self.nc = nc
        self.n_cores = n_cores
        partition_name = (
            nc.partition_id_tensor.name if nc.partition_id_tensor else None
        )
        in_names, out_names, out_avals, zero_outs = [], [], [], []
        for alloc in nc.m.functions[0].allocations:
            if not isinstance(alloc, mybir.MemoryLocationSet):
                continue
            name = alloc.memorylocations[0].name
            if alloc.kind == "ExternalInput":
                if name != partition_name:
                    in_names.append(name)
            elif alloc.kind == "ExternalOutput":
                shape = tuple(alloc.tensor_shape)
                dtype = mybir.dt.np(alloc.dtype)
                out_names.append(name)
                out_avals.append(jax.core.ShapedArray(shape, dtype))
                zero_outs.append(np.zeros(shape, dtype))
        self.in_names, self.out_names = in_names, out_names
        self.out_avals, self.zero_outs = out_avals, zero_outs
        n_params, n_outs = len(in_names), len(out_names)
        all_in_names = in_names + out_names
        if partition_name is not None:
            all_in_names.append(partition_name)
        donate = tuple(range(n_params, n_params + n_outs))

        def _body(*args):
            operands = list(args)
            if partition_name is not None:
                operands.append(bass2jax.partition_id_tensor())
            return tuple(
                _bass_exec_p.bind(
                    *operands,
                    out_avals=tuple(out_avals),
                    in_names=tuple(all_in_names),
                    out_names=tuple(out_names),
                    lowering_input_output_aliases=(),
                    sim_require_finite=True,
                    sim_require_nnan=True,
                    nc=nc,
                )
            )

        devices = jax.devices()[:n_cores]
        mesh = Mesh(np.asarray(devices), ("core",))
        in_specs = (PartitionSpec("core"),) * (n_params + n_outs)
        out_specs = (PartitionSpec("core"),) * n_outs
        self.fn = jax.jit(
            shard_map(
                _body,
                mesh=mesh,
                in_specs=in_specs,
                out_specs=out_specs,
                check_rep=False,
            ),
            donate_argnums=donate,
            keep_unused=True,
        )

    def run(self, in_maps):
        n = self.n_cores
        concat_in = [
            np.concatenate(
                [np.asarray(in_maps[c][name]) for c in range(n)], axis=0
            )
            for name in self.in_names
        ]
        zeros = [
            np.zeros((n * z.shape[0], *z.shape[1:]), z.dtype)
            for z in self.zero_outs
        ]
        out_arrs = self.fn(*concat_in, *zeros)
        return [
            {
                name: np.asarray(out_arrs[i]).reshape(
                    n, *self.out_avals[i].shape
                )[c]
                for i, name in enumerate(self.out_names)
            }
            for c in range(n)
        ]


_RUNNER = None


def _get_runner():
    global _RUNNER
    if _RUNNER is None:
        nc = _build_nc()
        _legalize(nc)
        _RUNNER = _Runner(nc, 8)
    return _RUNNER


# ---------------------------------------------------------------------------
# public entry point
# ---------------------------------------------------------------------------
def kernel(x, Wqkv, Wproj):
    x = np.asarray(x, dtype=np.float32)
    Wqkv = np.asarray(Wqkv, dtype=np.float32)
    Wproj = np.asarray(Wproj, dtype=np.float32)
    bf = ml_dtypes.bfloat16
    perm = _dim_perm()

    xT = [np.ascontiguousarray(x[b].T).astype(bf) for b in range(B)]
    in_maps = []
    for c in range(8):
        b, g = c // 4, c % 4
        heads = range(NH_CORE * g, NH_CORE * (g + 1))
        qcols = np.concatenate([h * HD + perm for h in heads])
        Wq_c = Wqkv[:, 0 * D + qcols].astype(bf)
        Wk_c = Wqkv[:, 1 * D + qcols].astype(bf)
        Wv_c = Wqkv[:, 2 * D + g * HCOLS : 2 * D + (g + 1) * HCOLS].astype(bf)
        Wp_c = Wproj[g * HCOLS : (g + 1) * HCOLS, :].astype(bf)
        in_maps.append(
            {"xT": xT[b], "Wq": Wq_c, "Wk": Wk_c, "Wv": Wv_c, "Wp": Wp_c}
        )

    results = _get_runner().run(in_maps)
    out = np.zeros((B, S, D), dtype=np.float32)
    for c in range(8):
        out[c // 4] += results[c]["out"]
    return out



# revision 30
# speedup vs baseline: 1.2349x; 1.2349x over previous
"""Trainium2 Bass kernel for nn_MultiHeadAttention (dense transformer block:
qkv proj + RoPE + causal SDPA + out proj), tensor-parallel over (batch, heads)
across 8 NeuronCores.

Sharding: 2 batches x 16 heads = 32 (b,h) pairs; core c handles batch c//4,
heads 4*(c%4)..4*(c%4)+3. Each core computes qkv for its 4 heads (from the
full x of its batch), RoPE, causal attention, and a PARTIAL output
projection (its heads' rows of Wproj); the host sums the 4 partials per
batch.

Perf design (CoreSim cost model: matmul = out_free x cycles_per_row):
- All projection GEMMs (qkv + out proj) run in fp8e4 with DoubleRow perf
  mode (2 k-tiles per instruction at 0.5 cycles/row = 4x bf16 throughput).
  Accuracy is recovered with a 3-term hi/lo error-compensated product:
  x@W ~= x_hi@W_hi + x_hi@W_lo + x_lo@W_hi, where *_lo are fp8 raw
  residuals accumulated in the same PSUM (scales baked host-side: W
  pre-scaled by WS, undone in the RoPE tables / V copy / out copy).
- Attention runs in fp16 (1 cycle/row, better mantissa than bf16):
  scores^T via K^T@Q per 128-kv-block, exp on ACT with a -ln16 bias
  (fp16-safe range), causal triangle zeroed post-exp by a Pool-engine
  affine_select, O = V^T@P^T and the softmax denominator via a
  ones-matmul, both PSUM-accumulated over kv blocks. ACT paces the
  attention window (the exp stream), so the Y fp8-split copies run on
  DVE, keeping ACT for exp only.
- Phase-major schedule: all projections (qc-major across heads so the 8
  PSUM groups of one token window fill while the next window's x streams
  in), then attention, then the out projection. DMA transfers serialize
  per issuing engine ring (SP/ACT/Pool) and are laid out so arrival order
  matches consumption order; output partials are written as fp16 (summed
  in fp32 on the host) to halve output DMA.
"""
import sys

sys.path.insert(0, "/opt/trn_rl_repo")

import numpy as np
import ml_dtypes

import concourse.bass as bass
import concourse.mybir as mybir
import concourse.tile as tile

P = 128
B, S, D = 2, 2048, 2048
NH, HD = 16, 128
NH_CORE = 4  # heads per core
HCOLS = NH_CORE * HD  # 512
KT = D // P  # 16 k-tiles
NKP = KT // 2  # 8 k-tile pairs (DoubleRow)
TT = S // P  # 16 token tiles
QC = 512  # q-chunk width
NQC = S // QC  # 4
ROPE_THETA = 10000.0
SCALE = HD**-0.5
WS = 32.0  # host-side weight scale (fp8 dynamic range)
YS = 16.0  # Y scale before fp8 split
LN16 = 2.772588722239781  # ln(16): exp bias keeps P, l in fp16 range

F32 = mybir.dt.float32
F16 = mybir.dt.float16
FP8 = mybir.dt.float8e4
E4 = ml_dtypes.float8_e4m3
DR = mybir.MatmulPerfMode.DoubleRow

_SWAP16 = [(i + 16) % 32 for i in range(32)]


# ---------------------------------------------------------------------------
# host-side constant tables
# ---------------------------------------------------------------------------
def _dim_perm():
    """Permutation p -> original head-dim index, 16-interleaved even/odd."""
    perm = np.zeros(HD, dtype=np.int64)
    for p in range(HD):
        qd, sl = p // 32, p % 32
        i = 16 * qd + (sl % 16)
        perm[p] = 2 * i if sl < 16 else 2 * i + 1
    return perm


def _rope_tables():
    """ctab[p,t], stab[p,t] (sign-baked) for the permuted head-dim layout."""
    inv_freq = 1.0 / (ROPE_THETA ** (np.arange(0, HD, 2, dtype=np.float64) / HD))
    t = np.arange(S, dtype=np.float64)
    ctab = np.zeros((HD, S), dtype=np.float64)
    stab = np.zeros((HD, S), dtype=np.float64)
    for p in range(HD):
        qd, sl = p // 32, p % 32
        i = 16 * qd + (sl % 16)
        ang = t * inv_freq[i]
        ctab[p] = np.cos(ang)
        stab[p] = -np.sin(ang) if sl < 16 else np.sin(ang)
    return ctab, stab


def _split8(a):
    hi = a.astype(E4)
    lo = (a - hi.astype(np.float32)).astype(E4)
    return hi, lo


# ---------------------------------------------------------------------------
# device kernel
# ---------------------------------------------------------------------------
def _build_nc():
    nc = bass.Bass()

    xhi = nc.declare_dram_parameter("xhi", [D, S], FP8, isOutput=False)
    xlo = nc.declare_dram_parameter("xlo", [D, S], FP8, isOutput=False)
    wq_hi = nc.declare_dram_parameter("wq_hi", [D, HCOLS], FP8, isOutput=False)
    wq_lo = nc.declare_dram_parameter("wq_lo", [D, HCOLS], FP8, isOutput=False)
    wk_hi = nc.declare_dram_parameter("wk_hi", [D, HCOLS], FP8, isOutput=False)
    wk_lo = nc.declare_dram_parameter("wk_lo", [D, HCOLS], FP8, isOutput=False)
    wv_hi = nc.declare_dram_parameter("wv_hi", [D, HCOLS], FP8, isOutput=False)
    wv_lo = nc.declare_dram_parameter("wv_lo", [D, HCOLS], FP8, isOutput=False)
    wp_hi = nc.declare_dram_parameter("wp_hi", [HCOLS, D], FP8, isOutput=False)
    wp_lo = nc.declare_dram_parameter("wp_lo", [HCOLS, D], FP8, isOutput=False)
    out = nc.declare_dram_parameter("out", [S, D], F16, isOutput=True)

    # sqrt(SCALE)/WS on both q and k tables => scores scaled by SCALE and the
    # host-side WS weight scale undone.
    ctab_np, stab_np = _rope_tables()
    rt = np.sqrt(SCALE) / WS
    cq_d = nc.inline_tensor((ctab_np * rt).astype(np.float16), "cq")
    sq_d = nc.inline_tensor((stab_np * rt).astype(np.float16), "sq")

    xhi_t = xhi[:].rearrange("(ko p) t -> p ko t", p=P)
    xlo_t = xlo[:].rearrange("(ko p) t -> p ko t", p=P)
    w_t = {
        n: w[:].rearrange("(ko p) m -> p ko m", p=P)
        for n, w in (
            ("q_hi", wq_hi), ("q_lo", wq_lo),
            ("k_hi", wk_hi), ("k_lo", wk_lo),
            ("v_hi", wv_hi), ("v_lo", wv_lo),
        )
    }
    wp_hi_t = wp_hi[:].rearrange("(ho p) n -> p ho n", p=P)
    wp_lo_t = wp_lo[:].rearrange("(ho p) n -> p ho n", p=P)
    out_t = out[:].rearrange("(to p) n -> p to n", p=P)

    with tile.TileContext(nc) as tc:
        with tc.tile_pool(name="persist", bufs=1) as pp:
            # persistent tiles
            cq = pp.tile([P, S], F16)
            sq = pp.tile([P, S], F16)
            ebias = pp.tile([P, 1], F32)
            nc.vector.memset(ebias, -LN16)
            ones_sb = pp.tile([P, P], F16)
            nc.vector.memset(ones_sb, 1.0)
            zfill = nc.gpsimd.to_reg(0.0)

            Vt = pp.tile([P, TT, HCOLS], F16)
            Qt = pp.tile([P, NH_CORE, S], F16)
            Kt = pp.tile([P, NH_CORE, S], F16)
            Yhi = pp.tile([P, NH_CORE, S], FP8)
            Ylo = pp.tile([P, NH_CORE, S], FP8)

            # ------------- phase 1: qkv projections + RoPE ----------------
            with (
                tc.tile_pool(name="xw", bufs=1) as xw,
                tc.tile_pool(name="wk1", bufs=2) as wk,
                tc.tile_pool(name="ps_a", bufs=8, space="PSUM") as psA,
            ):
                KH = KT // 2
                xhi_sb = [
                    [xw.tile([P, KH, QC], FP8, name=f"xhi_{w}_{hh}") for hh in (0, 1)]
                    for w in range(NQC)
                ]
                xlo_sb = [
                    [xw.tile([P, KH, QC], FP8, name=f"xlo_{w}_{hh}") for hh in (0, 1)]
                    for w in range(NQC)
                ]
                # q/k weights as separate half-K tiles so the first
                # projection groups depend only on the first half-transfer
                w_sb = {}
                for n in w_t:
                    if n.startswith("v"):
                        w_sb[n] = xw.tile([P, KT, HCOLS], FP8, name=f"w_{n}")
                    else:
                        w_sb[n] = [
                            xw.tile([P, KT // 2, HCOLS], FP8, name=f"w_{n}_{hh}")
                            for hh in (0, 1)
                        ]
                # All DMA transfers serialize on one global DMA device, so
                # ARRIVAL ORDER is the schedule: q/k weights first, then x
                # streamed by token window (every projection group of a
                # window becomes runnable as soon as that window lands).
                # DMA transfers serialize per issuing engine but run in
                # parallel across engines: split the x windows over the SP
                # and ACT rings (ACT's issues finish before its first copy
                # work), weights/tables on the Pool ring.
                for qcx in range(NQC):
                    win = slice(qcx * QC, (qcx + 1) * QC)
                    eng = nc.sync if qcx % 2 == 0 else nc.scalar
                    for hh in (0, 1):
                        ks = slice(hh * KH, (hh + 1) * KH)
                        eng.dma_start(xhi_sb[qcx][hh], xhi_t[:, ks, win])
                    for hh in (0, 1):
                        ks = slice(hh * KH, (hh + 1) * KH)
                        eng.dma_start(xlo_sb[qcx][hh], xlo_t[:, ks, win])
                for hh in (0, 1):
                    ks = slice(hh * (KT // 2), (hh + 1) * (KT // 2))
                    nc.gpsimd.dma_start(w_sb["q_hi"][hh], w_t["q_hi"][:, ks])
                    nc.gpsimd.dma_start(w_sb["q_lo"][hh], w_t["q_lo"][:, ks])
                for hh in (0, 1):
                    ks = slice(hh * (KT // 2), (hh + 1) * (KT // 2))
                    nc.gpsimd.dma_start(w_sb["k_hi"][hh], w_t["k_hi"][:, ks])
                    nc.gpsimd.dma_start(w_sb["k_lo"][hh], w_t["k_lo"][:, ks])
                nc.gpsimd.dma_start(cq, cq_d[:])
                nc.gpsimd.dma_start(sq, sq_d[:])
                nc.gpsimd.dma_start(w_sb["v_hi"], w_t["v_hi"])
                nc.gpsimd.dma_start(w_sb["v_lo"], w_t["v_lo"])

                def proj_tile(ps, pre, qcx, mcols):
                    """24 DR matmuls: 3-term fp8 product into one psum tile."""
                    terms = (
                        (w_sb[f"{pre}_hi"], xhi_sb),
                        (w_sb[f"{pre}_lo"], xhi_sb),
                        (w_sb[f"{pre}_hi"], xlo_sb),
                    )
                    for i, (wt, xt) in enumerate(terms):
                        for t in range(NKP):
                            hh, tl = t // (NKP // 2), t % (NKP // 2)
                            ksl = slice(2 * tl, 2 * tl + 2)
                            nc.tensor.matmul(
                                ps,
                                wt[hh][:, ksl, mcols],
                                xt[qcx][hh][:, ksl],
                                start=(i == 0 and t == 0),
                                stop=(i == 2 and t == NKP - 1),
                                perf_mode=DR,
                            )

                # qc-major across heads: the 8 groups of one token window
                # (4 heads x q/k) fill all 8 PSUM banks while the next
                # window streams in.
                for qcx in range(NQC):
                    win = slice(qcx * QC, (qcx + 1) * QC)
                    for h in range(NH_CORE):
                        hs = slice(h * HD, (h + 1) * HD)
                        for pre, O_t in (("q", Qt), ("k", Kt)):
                            ps = psA.tile([P, QC], F32, tag="ps")
                            proj_tile(ps, pre, qcx, hs)
                            pc = wk.tile([P, QC], F16, tag="pc")
                            nc.scalar.activation(
                                pc, ps, mybir.ActivationFunctionType.Copy
                            )
                            xsw = wk.tile([P, QC], F16, tag="xsw")
                            nc.vector.stream_shuffle(xsw, pc, _SWAP16)
                            m1 = wk.tile([P, QC], F16, tag="m1")
                            nc.vector.tensor_mul(m1, pc, cq[:, win])
                            m2 = wk.tile([P, QC], F16, tag="m2")
                            nc.vector.tensor_mul(m2, xsw, sq[:, win])
                            nc.vector.tensor_add(O_t[:, h, win], m1, m2)

                # v projection (token-major, all heads)
                for tt in range(TT):
                    wq_, to_ = tt // 4, (tt % 4) * P
                    tw = slice(to_, to_ + P)
                    ps = psA.tile([P, HCOLS], F32, tag="ps")
                    terms = (
                        (w_sb["v_hi"], xhi_sb),
                        (w_sb["v_lo"], xhi_sb),
                        (w_sb["v_hi"], xlo_sb),
                    )
                    for i, (wt, xt) in enumerate(terms):
                        for t in range(NKP):
                            ks = slice(2 * t, 2 * t + 2)
                            hh, tl = t // (NKP // 2), t % (NKP // 2)
                            ksl = slice(2 * tl, 2 * tl + 2)
                            nc.tensor.matmul(
                                ps,
                                xt[wq_][hh][:, ksl, tw],
                                wt[:, ks],
                                start=(i == 0 and t == 0),
                                stop=(i == 2 and t == NKP - 1),
                                perf_mode=DR,
                            )
                    nc.scalar.activation(
                        Vt[:, tt],
                        ps,
                        mybir.ActivationFunctionType.Copy,
                        scale=1.0 / WS,
                    )

            # ------------- phases 2+3: attention, out projection ----------
            with tc.tile_pool(name="outw", bufs=1) as oww:
                # out-proj weights go into the space freed by phase 1
                wp_hi_sb = oww.tile([P, NH_CORE, D], FP8)
                wp_lo_sb = oww.tile([P, NH_CORE, D], FP8)
                nc.sync.dma_start(wp_hi_sb, wp_hi_t)
                nc.sync.dma_start(wp_lo_sb, wp_lo_t)

                AC = 512  # attention q-window (one PSUM bank fp32)
                NAC = S // AC
                with (
                    tc.tile_pool(name="wk2", bufs=3) as wk2,
                    tc.tile_pool(name="pt", bufs=6) as ptp,
                    tc.tile_pool(name="ps_s", bufs=4, space="PSUM") as psS,
                    tc.tile_pool(name="ps_o", bufs=2, space="PSUM") as psO,
                    tc.tile_pool(name="ps_l", bufs=2, space="PSUM") as psL,
                ):
                    for h in range(NH_CORE):
                        hs = slice(h * HD, (h + 1) * HD)
                        for qcx in range(NAC):
                            win = slice(qcx * AC, (qcx + 1) * AC)
                            o_ps = psO.tile([P, AC], F32, tag="ops")
                            l_ps = psL.tile([P, AC], F32, tag="lps")
                            njb = 4 * qcx + 4
                            for jb in range(njb):
                                d = jb - 4 * qcx  # diag offset if >= 0
                                off = 128 * d if d > 0 else 0
                                s_ps = psS.tile([P, AC], F32, tag="sps")
                                nc.tensor.matmul(
                                    s_ps[:, off:],
                                    Kt[:, h, jb * P : (jb + 1) * P],
                                    Qt[:, h, qcx * AC + off : (qcx + 1) * AC],
                                    start=True,
                                    stop=True,
                                )
                                pt = ptp.tile([P, AC], F16, tag="pt")
                                nc.scalar.activation(
                                    pt[:, off:],
                                    s_ps[:, off:],
                                    mybir.ActivationFunctionType.Exp,
                                    bias=ebias,
                                )
                                if d >= 0:
                                    # zero the upper triangle (kv > q) post-exp
                                    nc.gpsimd.affine_select(
                                        pt[:, off : off + P],
                                        pt[:, off : off + P],
                                        pattern=[[1, P]],
                                        compare_op=mybir.AluOpType.is_ge,
                                        fill=zfill,
                                        base=0,
                                        channel_multiplier=-1,
                                    )
                                nc.tensor.matmul(
                                    o_ps[:, off:],
                                    Vt[:, jb, hs],
                                    pt[:, off:],
                                    start=(jb == 0),
                                    stop=(jb == njb - 1),
                                )
                                nc.tensor.matmul(
                                    l_ps[:, off:],
                                    ones_sb,
                                    pt[:, off:],
                                    start=(jb == 0),
                                    stop=(jb == njb - 1),
                                )
                            rinv = wk2.tile([P, AC], F32, tag="rinv")
                            nc.vector.reciprocal(rinv, l_ps)
                            # yt = (o_ps * YS) * rinv, fp16 (Y scaled by YS
                            # for the fp8 split; undone in the out-proj copy)
                            yt = wk2.tile([P, AC], F16, tag="yt")
                            nc.vector.scalar_tensor_tensor(
                                yt, o_ps, YS, rinv,
                                op0=mybir.AluOpType.mult,
                                op1=mybir.AluOpType.mult,
                            )
                            nc.vector.tensor_copy(Yhi[:, h, win], yt)
                            nc.vector.scalar_tensor_tensor(
                                Ylo[:, h, win], Yhi[:, h, win], -1.0, yt,
                                op0=mybir.AluOpType.mult,
                                op1=mybir.AluOpType.add,
                            )

                # ------------ phase 3: out projection (fp8 DR) -------------
                with (
                    tc.tile_pool(name="outp", bufs=2) as outp,
                    tc.tile_pool(name="ps_p", bufs=4, space="PSUM") as psP,
                ):
                    for tt in range(TT):
                        tw = slice(tt * P, (tt + 1) * P)
                        ob = outp.tile([P, D], F16, tag="ob")
                        for ncx in range(D // QC):
                            nwin = slice(ncx * QC, (ncx + 1) * QC)
                            ps = psP.tile([P, QC], F32, tag="psp")
                            terms = (
                                (Yhi, wp_hi_sb),
                                (Yhi, wp_lo_sb),
                                (Ylo, wp_hi_sb),
                            )
                            for i, (yt8, wt8) in enumerate(terms):
                                for hp in range(NH_CORE // 2):
                                    ks = slice(2 * hp, 2 * hp + 2)
                                    nc.tensor.matmul(
                                        ps,
                                        yt8[:, ks, tw],
                                        wt8[:, ks, nwin],
                                        start=(i == 0 and hp == 0),
                                        stop=(i == 2 and hp == NH_CORE // 2 - 1),
                                        perf_mode=DR,
                                    )
                            if ncx % 2 == 0:
                                nc.scalar.activation(
                                    ob[:, nwin],
                                    ps,
                                    mybir.ActivationFunctionType.Copy,
                                    scale=1.0 / (WS * YS),
                                )
                            else:
                                nc.vector.tensor_scalar_mul(
                                    ob[:, nwin], ps, 1.0 / (WS * YS)
                                )
                        if tt < TT - 1:
                            nc.sync.dma_start(
                                out_t[:, tt, 0:1024], ob[:, 0:1024]
                            )
                            nc.gpsimd.dma_start(
                                out_t[:, tt, 1024:2048], ob[:, 1024:2048]
                            )
                        else:
                            # final tile: small chunks so the drain is short
                            for qi, eng in enumerate(
                                (nc.sync, nc.gpsimd, nc.sync, nc.gpsimd)
                            ):
                                cw = slice(qi * QC, (qi + 1) * QC)
                                eng.dma_start(out_t[:, tt, cw], ob[:, cw])
    return nc


# ---------------------------------------------------------------------------
# legalization: this walrus build supports only ONE sync wait per instruction
# ---------------------------------------------------------------------------
_ENGINE_SEM_PREFIX = {
    "PE": "PE_",
    "DVE": "DVE_",
    "ACT": "ACT_",
    "Pool": "POOL_",
    "SP": "SP_",
}
_wf_counter = [0]


def _legalize(nc, max_waits=1):
    for f in nc.m.functions:
        for bb in f.blocks:
            new_insts = []
            for inst in bb.instructions:
                si = getattr(inst, "sync_info", None)
                eng = getattr(inst, "engine", None)
                if si is None or not si.on_wait or eng is None:
                    new_insts.append(inst)
                    continue
                waits = list(si.on_wait)
                pref = _ENGINE_SEM_PREFIX.get(eng.name)
                if pref is not None:
                    waits = [
                        w
                        for w in waits
                        if not (
                            w.sync_type == "semaphore"
                            and w.ant_name.startswith(pref)
                        )
                    ]
                if len(waits) > max_waits:
                    for w in waits[:-max_waits]:
                        _wf_counter[0] += 1
                        nop = mybir.InstNoOp(
                            name=f"I-waitfix-{_wf_counter[0]}", ins=[], outs=[]
                        )
                        nop.engine = eng
                        nop.sync_info = mybir.SyncInfo(on_wait=[w], on_update=[])
                        new_insts.append(nop)
                    waits = waits[-max_waits:]
                if len(waits) != len(si.on_wait):
                    inst.sync_info = mybir.SyncInfo(
                        on_wait=waits, on_update=list(si.on_update)
                    )
                new_insts.append(inst)
            bb.instructions[:] = new_insts


# ---------------------------------------------------------------------------
# SPMD runner (mirrors concourse.bass2jax.run_bass_via_pjrt, kept resident)
# ---------------------------------------------------------------------------
class _Runner:
    def __init__(self, nc, n_cores=8):
        import jax
        from jax.sharding import Mesh, PartitionSpec
        from jax.experimental.shard_map import shard_map
        from concourse import bass2jax
        from concourse.bass2jax import _bass_exec_p, install_neuronx_cc_hook

        install_neuronx_cc_hook()
        self.jax = jax
        self.nc = nc
        self.n_cores = n_cores
        partition_name = (
            nc.partition_id_tensor.name if nc.partition_id_tensor else None
        )
        in_names, out_names, out_avals, zero_outs = [], [], [], []
        for alloc in nc.m.functions[0].allocations:
            if not isinstance(alloc, mybir.MemoryLocationSet):
                continue
            name = alloc.memorylocations[0].name
            if alloc.kind == "ExternalInput":
                if name != partition_name:
                    in_names.append(name)
            elif alloc.kind == "ExternalOutput":
                shape = tuple(alloc.tensor_shape)
                dtype = mybir.dt.np(alloc.dtype)
                out_names.append(name)
                out_avals.append(jax.core.ShapedArray(shape, dtype))
                zero_outs.append(np.zeros(shape, dtype))
        self.in_names, self.out_names = in_names, out_names
        self.out_avals, self.zero_outs = out_avals, zero_outs
        n_params, n_outs = len(in_names), len(out_names)
        all_in_names = in_names + out_names
        if partition_name is not None:
            all_in_names.append(partition_name)
        donate = tuple(range(n_params, n_params + n_outs))

        def _body(*args):
            operands = list(args)
            if partition_name is not None:
                operands.append(bass2jax.partition_id_tensor())
            return tuple(
                _bass_exec_p.bind(
                    *operands,
                    out_avals=tuple(out_avals),
                    in_names=tuple(all_in_names),
                    out_names=tuple(out_names),
                    lowering_input_output_aliases=(),
                    sim_require_finite=True,
                    sim_require_nnan=True,
                    nc=nc,
                )
            )

        devices = jax.devices()[:n_cores]
        mesh = Mesh(np.asarray(devices), ("core",))
        in_specs = (PartitionSpec("core"),) * (n_params + n_outs)
        out_specs = (PartitionSpec("core"),) * n_outs
        self.fn = jax.jit(
            shard_map(
                _body,
                mesh=mesh,
                in_specs=in_specs,
                out_specs=out_specs,
                check_rep=False,
            ),
            donate_argnums=donate,
            keep_unused=True,
        )

    def run(self, in_maps):
        n = self.n_cores
        concat_in = [
            np.concatenate(
                [np.asarray(in_maps[c][name]) for c in range(n)], axis=0
            )
            for name in self.in_names
        ]
        zeros = [
            np.zeros((n * z.shape[0], *z.shape[1:]), z.dtype)
            for z in self.zero_outs
        ]
        out_arrs = self.fn(*concat_in, *zeros)
        return [
            {
                name: np.asarray(out_arrs[i]).reshape(
                    n, *self.out_avals[i].shape
                )[c]
                for i, name in enumerate(self.out_names)
            }
            for c in range(n)
        ]


_RUNNER = None


def _get_runner():
    global _RUNNER
    if _RUNNER is None:
        nc = _build_nc()
        _legalize(nc)
        _RUNNER = _Runner(nc, 8)
    return _RUNNER


# ---------------------------------------------------------------------------
# public entry point
# ---------------------------------------------------------------------------
def kernel(x, Wqkv, Wproj):
    x = np.asarray(x, dtype=np.float32)
    Wqkv = np.asarray(Wqkv, dtype=np.float32)
    Wproj = np.asarray(Wproj, dtype=np.float32)
    perm = _dim_perm()

    xT8 = [
        _split8(np.ascontiguousarray(x[b].T)) for b in range(B)
    ]  # (hi, lo) per batch
    in_maps = []
    for c in range(8):
        b, g = c // 4, c % 4
        heads = range(NH_CORE * g, NH_CORE * (g + 1))
        qcols = np.concatenate([h * HD + perm for h in heads])
        wq = _split8(Wqkv[:, 0 * D + qcols] * WS)
        wkk = _split8(Wqkv[:, 1 * D + qcols] * WS)
        wv = _split8(
            Wqkv[:, 2 * D + g * HCOLS : 2 * D + (g + 1) * HCOLS] * WS
        )
        wp = _split8(Wproj[g * HCOLS : (g + 1) * HCOLS, :] * WS)
        in_maps.append(
            {
                "xhi": xT8[b][0], "xlo": xT8[b][1],
                "wq_hi": wq[0], "wq_lo": wq[1],
                "wk_hi": wkk[0], "wk_lo": wkk[1],
                "wv_hi": wv[0], "wv_lo": wv[1],
                "wp_hi": wp[0], "wp_lo": wp[1],
            }
        )

    results = _get_runner().run(in_maps)
    out = np.zeros((B, S, D), dtype=np.float32)
    for c in range(8):
        out[c // 4] += results[c]["out"].astype(np.float32)
    return out


# revision 31
# speedup vs baseline: 1.2386x; 1.0030x over previous
"""Trainium2 Bass kernel for nn_MultiHeadAttention (dense transformer block:
qkv proj + RoPE + causal SDPA + out proj), tensor-parallel over (batch, heads)
across 8 NeuronCores.

Sharding: 2 batches x 16 heads = 32 (b,h) pairs; core c handles batch c//4,
heads 4*(c%4)..4*(c%4)+3. Each core computes qkv for its 4 heads (from the
full x of its batch), RoPE, causal attention, and a PARTIAL output
projection (its heads' rows of Wproj); the host sums the 4 partials per
batch.

Perf design (CoreSim cost model: matmul = out_free x cycles_per_row):
- All projection GEMMs (qkv + out proj) run in fp8e4 with DoubleRow perf
  mode (2 k-tiles per instruction at 0.5 cycles/row = 4x bf16 throughput).
  Accuracy is recovered with a 3-term hi/lo error-compensated product:
  x@W ~= x_hi@W_hi + x_hi@W_lo + x_lo@W_hi, where *_lo are fp8 raw
  residuals accumulated in the same PSUM (scales baked host-side: W
  pre-scaled by WS, undone in the RoPE tables / V copy / out copy).
- Attention runs in fp16 (1 cycle/row, better mantissa than bf16):
  scores^T via K^T@Q per 128-kv-block, exp on ACT with a -ln16 bias
  (fp16-safe range), causal triangle zeroed post-exp by a Pool-engine
  affine_select, O = V^T@P^T and the softmax denominator via a
  ones-matmul, both PSUM-accumulated over kv blocks. ACT paces the
  attention window (the exp stream), so the Y fp8-split copies run on
  DVE, keeping ACT for exp only.
- Phase-major schedule: all projections (qc-major across heads so the 8
  PSUM groups of one token window fill while the next window's x streams
  in), then attention, then the out projection. DMA transfers serialize
  per issuing engine ring (SP/ACT/Pool) and are laid out so arrival order
  matches consumption order; output partials are written as fp16 (summed
  in fp32 on the host) to halve output DMA.
"""
import sys

sys.path.insert(0, "/opt/trn_rl_repo")

import numpy as np
import ml_dtypes

import concourse.bass as bass
import concourse.mybir as mybir
import concourse.tile as tile

P = 128
B, S, D = 2, 2048, 2048
NH, HD = 16, 128
NH_CORE = 4  # heads per core
HCOLS = NH_CORE * HD  # 512
KT = D // P  # 16 k-tiles
NKP = KT // 2  # 8 k-tile pairs (DoubleRow)
TT = S // P  # 16 token tiles
QC = 512  # q-chunk width
NQC = S // QC  # 4
ROPE_THETA = 10000.0
SCALE = HD**-0.5
WS = 32.0  # host-side weight scale (fp8 dynamic range)
YS = 16.0  # Y scale before fp8 split
LN16 = 2.772588722239781  # ln(16): exp bias keeps P, l in fp16 range

F32 = mybir.dt.float32
F16 = mybir.dt.float16
FP8 = mybir.dt.float8e4
E4 = ml_dtypes.float8_e4m3
DR = mybir.MatmulPerfMode.DoubleRow

_SWAP16 = [(i + 16) % 32 for i in range(32)]


# ---------------------------------------------------------------------------
# host-side constant tables
# ---------------------------------------------------------------------------
def _dim_perm():
    """Permutation p -> original head-dim index, 16-interleaved even/odd."""
    perm = np.zeros(HD, dtype=np.int64)
    for p in range(HD):
        qd, sl = p // 32, p % 32
        i = 16 * qd + (sl % 16)
        perm[p] = 2 * i if sl < 16 else 2 * i + 1
    return perm


def _rope_tables():
    """ctab[p,t], stab[p,t] (sign-baked) for the permuted head-dim layout."""
    inv_freq = 1.0 / (ROPE_THETA ** (np.arange(0, HD, 2, dtype=np.float64) / HD))
    t = np.arange(S, dtype=np.float64)
    ctab = np.zeros((HD, S), dtype=np.float64)
    stab = np.zeros((HD, S), dtype=np.float64)
    for p in range(HD):
        qd, sl = p // 32, p % 32
        i = 16 * qd + (sl % 16)
        ang = t * inv_freq[i]
        ctab[p] = np.cos(ang)
        stab[p] = -np.sin(ang) if sl < 16 else np.sin(ang)
    return ctab, stab


def _split8(a):
    hi = a.astype(E4)
    lo = (a - hi.astype(np.float32)).astype(E4)
    return hi, lo


# ---------------------------------------------------------------------------
# device kernel
# ---------------------------------------------------------------------------
def _build_nc():
    nc = bass.Bass()

    xhi = nc.declare_dram_parameter("xhi", [D, S], FP8, isOutput=False)
    xlo = nc.declare_dram_parameter("xlo", [D, S], FP8, isOutput=False)
    wq_hi = nc.declare_dram_parameter("wq_hi", [D, HCOLS], FP8, isOutput=False)
    wq_lo = nc.declare_dram_parameter("wq_lo", [D, HCOLS], FP8, isOutput=False)
    wk_hi = nc.declare_dram_parameter("wk_hi", [D, HCOLS], FP8, isOutput=False)
    wk_lo = nc.declare_dram_parameter("wk_lo", [D, HCOLS], FP8, isOutput=False)
    wv_hi = nc.declare_dram_parameter("wv_hi", [D, HCOLS], FP8, isOutput=False)
    wv_lo = nc.declare_dram_parameter("wv_lo", [D, HCOLS], FP8, isOutput=False)
    wp_hi = nc.declare_dram_parameter("wp_hi", [HCOLS, D], FP8, isOutput=False)
    wp_lo = nc.declare_dram_parameter("wp_lo", [HCOLS, D], FP8, isOutput=False)
    out = nc.declare_dram_parameter("out", [S, D], F16, isOutput=True)

    # sqrt(SCALE)/WS on both q and k tables => scores scaled by SCALE and the
    # host-side WS weight scale undone.
    ctab_np, stab_np = _rope_tables()
    rt = np.sqrt(SCALE) / WS
    cq_d = nc.inline_tensor((ctab_np * rt).astype(np.float16), "cq")
    sq_d = nc.inline_tensor((stab_np * rt).astype(np.float16), "sq")

    xhi_t = xhi[:].rearrange("(ko p) t -> p ko t", p=P)
    xlo_t = xlo[:].rearrange("(ko p) t -> p ko t", p=P)
    w_t = {
        n: w[:].rearrange("(ko p) m -> p ko m", p=P)
        for n, w in (
            ("q_hi", wq_hi), ("q_lo", wq_lo),
            ("k_hi", wk_hi), ("k_lo", wk_lo),
            ("v_hi", wv_hi), ("v_lo", wv_lo),
        )
    }
    wp_hi_t = wp_hi[:].rearrange("(ho p) n -> p ho n", p=P)
    wp_lo_t = wp_lo[:].rearrange("(ho p) n -> p ho n", p=P)
    out_t = out[:].rearrange("(to p) n -> p to n", p=P)

    with tile.TileContext(nc) as tc:
        with tc.tile_pool(name="persist", bufs=1) as pp:
            # persistent tiles
            cq = pp.tile([P, S], F16)
            sq = pp.tile([P, S], F16)
            ebias = pp.tile([P, 1], F32)
            nc.vector.memset(ebias, -LN16)
            ones_sb = pp.tile([P, P], F16)
            nc.vector.memset(ones_sb, 1.0)
            zfill = nc.gpsimd.to_reg(0.0)

            Vt = pp.tile([P, TT, HCOLS], F16)
            Qt = pp.tile([P, NH_CORE, S], F16)
            Kt = pp.tile([P, NH_CORE, S], F16)
            Yhi = pp.tile([P, NH_CORE, S], FP8)
            Ylo = pp.tile([P, NH_CORE, S], FP8)

            # ------------- phase 1: qkv projections + RoPE ----------------
            with (
                tc.tile_pool(name="xw", bufs=1) as xw,
                tc.tile_pool(name="wk1", bufs=2) as wk,
                tc.tile_pool(name="ps_a", bufs=8, space="PSUM") as psA,
            ):
                KH = KT // 2
                xhi_sb = [
                    [xw.tile([P, KH, QC], FP8, name=f"xhi_{w}_{hh}") for hh in (0, 1)]
                    for w in range(NQC)
                ]
                xlo_sb = [
                    [xw.tile([P, KH, QC], FP8, name=f"xlo_{w}_{hh}") for hh in (0, 1)]
                    for w in range(NQC)
                ]
                # q/k weights as separate half-K tiles so the first
                # projection groups depend only on the first half-transfer
                w_sb = {}
                for n in w_t:
                    if n.startswith("v"):
                        w_sb[n] = xw.tile([P, KT, HCOLS], FP8, name=f"w_{n}")
                    elif n.startswith("q"):
                        # quarter tiles: first groups start ~0.8us earlier
                        w_sb[n] = [
                            xw.tile([P, KT // 4, HCOLS], FP8, name=f"w_{n}_{hh}")
                            for hh in range(4)
                        ]
                    else:
                        w_sb[n] = [
                            xw.tile([P, KT // 2, HCOLS], FP8, name=f"w_{n}_{hh}")
                            for hh in (0, 1)
                        ]
                # All DMA transfers serialize on one global DMA device, so
                # ARRIVAL ORDER is the schedule: q/k weights first, then x
                # streamed by token window (every projection group of a
                # window becomes runnable as soon as that window lands).
                # DMA transfers serialize per issuing engine but run in
                # parallel across engines: split the x windows over the SP
                # and ACT rings (ACT's issues finish before its first copy
                # work), weights/tables on the Pool ring.
                for qcx in range(NQC):
                    win = slice(qcx * QC, (qcx + 1) * QC)
                    eng = nc.sync if qcx % 2 == 0 else nc.scalar
                    for hh in (0, 1):
                        ks = slice(hh * KH, (hh + 1) * KH)
                        eng.dma_start(xhi_sb[qcx][hh], xhi_t[:, ks, win])
                    for hh in (0, 1):
                        ks = slice(hh * KH, (hh + 1) * KH)
                        eng.dma_start(xlo_sb[qcx][hh], xlo_t[:, ks, win])
                for hh in range(4):
                    ks = slice(hh * (KT // 4), (hh + 1) * (KT // 4))
                    nc.gpsimd.dma_start(w_sb["q_hi"][hh], w_t["q_hi"][:, ks])
                    nc.gpsimd.dma_start(w_sb["q_lo"][hh], w_t["q_lo"][:, ks])
                for hh in (0, 1):
                    ks = slice(hh * (KT // 2), (hh + 1) * (KT // 2))
                    nc.gpsimd.dma_start(w_sb["k_hi"][hh], w_t["k_hi"][:, ks])
                    nc.gpsimd.dma_start(w_sb["k_lo"][hh], w_t["k_lo"][:, ks])
                nc.gpsimd.dma_start(cq, cq_d[:])
                nc.gpsimd.dma_start(sq, sq_d[:])
                nc.gpsimd.dma_start(w_sb["v_hi"], w_t["v_hi"])
                nc.gpsimd.dma_start(w_sb["v_lo"], w_t["v_lo"])

                def proj_tile(ps, pre, qcx, mcols):
                    """24 DR matmuls: 3-term fp8 product into one psum tile."""
                    terms = (
                        (w_sb[f"{pre}_hi"], xhi_sb),
                        (w_sb[f"{pre}_lo"], xhi_sb),
                        (w_sb[f"{pre}_hi"], xlo_sb),
                    )
                    nqw = len(w_sb[f"{pre}_hi"])  # 4 for q, 2 for k
                    for i, (wt, xt) in enumerate(terms):
                        for t in range(NKP):
                            hw_, tw_ = t // (NKP // nqw), t % (NKP // nqw)
                            kw = slice(2 * tw_, 2 * tw_ + 2)
                            hh, tl = t // (NKP // 2), t % (NKP // 2)
                            ksl = slice(2 * tl, 2 * tl + 2)
                            nc.tensor.matmul(
                                ps,
                                wt[hw_][:, kw, mcols],
                                xt[qcx][hh][:, ksl],
                                start=(i == 0 and t == 0),
                                stop=(i == 2 and t == NKP - 1),
                                perf_mode=DR,
                            )

                # qc-major across heads: the 8 groups of one token window
                # (4 heads x q/k) fill all 8 PSUM banks while the next
                # window streams in.
                for qcx in range(NQC):
                    win = slice(qcx * QC, (qcx + 1) * QC)
                    for h in range(NH_CORE):
                        hs = slice(h * HD, (h + 1) * HD)
                        for pre, O_t in (("q", Qt), ("k", Kt)):
                            ps = psA.tile([P, QC], F32, tag="ps")
                            proj_tile(ps, pre, qcx, hs)
                            pc = wk.tile([P, QC], F16, tag="pc")
                            nc.scalar.activation(
                                pc, ps, mybir.ActivationFunctionType.Copy
                            )
                            xsw = wk.tile([P, QC], F16, tag="xsw")
                            nc.vector.stream_shuffle(xsw, pc, _SWAP16)
                            m1 = wk.tile([P, QC], F16, tag="m1")
                            nc.vector.tensor_mul(m1, pc, cq[:, win])
                            m2 = wk.tile([P, QC], F16, tag="m2")
                            nc.vector.tensor_mul(m2, xsw, sq[:, win])
                            nc.vector.tensor_add(O_t[:, h, win], m1, m2)

                # v projection (token-major, all heads)
                for tt in range(TT):
                    wq_, to_ = tt // 4, (tt % 4) * P
                    tw = slice(to_, to_ + P)
                    ps = psA.tile([P, HCOLS], F32, tag="ps")
                    terms = (
                        (w_sb["v_hi"], xhi_sb),
                        (w_sb["v_lo"], xhi_sb),
                        (w_sb["v_hi"], xlo_sb),
                    )
                    for i, (wt, xt) in enumerate(terms):
                        for t in range(NKP):
                            ks = slice(2 * t, 2 * t + 2)
                            hh, tl = t // (NKP // 2), t % (NKP // 2)
                            ksl = slice(2 * tl, 2 * tl + 2)
                            nc.tensor.matmul(
                                ps,
                                xt[wq_][hh][:, ksl, tw],
                                wt[:, ks],
                                start=(i == 0 and t == 0),
                                stop=(i == 2 and t == NKP - 1),
                                perf_mode=DR,
                            )
                    nc.scalar.activation(
                        Vt[:, tt],
                        ps,
                        mybir.ActivationFunctionType.Copy,
                        scale=1.0 / WS,
                    )

            # ------------- phases 2+3: attention, out projection ----------
            with tc.tile_pool(name="outw", bufs=1) as oww:
                # out-proj weights go into the space freed by phase 1
                wp_hi_sb = oww.tile([P, NH_CORE, D], FP8)
                wp_lo_sb = oww.tile([P, NH_CORE, D], FP8)
                nc.sync.dma_start(wp_hi_sb, wp_hi_t)
                nc.sync.dma_start(wp_lo_sb, wp_lo_t)

                AC = 512  # attention q-window (one PSUM bank fp32)
                NAC = S // AC
                with (
                    tc.tile_pool(name="wk2", bufs=3) as wk2,
                    tc.tile_pool(name="pt", bufs=6) as ptp,
                    tc.tile_pool(name="ps_s", bufs=4, space="PSUM") as psS,
                    tc.tile_pool(name="ps_o", bufs=2, space="PSUM") as psO,
                    tc.tile_pool(name="ps_l", bufs=2, space="PSUM") as psL,
                ):
                    for h in range(NH_CORE):
                        hs = slice(h * HD, (h + 1) * HD)
                        for qcx in range(NAC):
                            win = slice(qcx * AC, (qcx + 1) * AC)
                            o_ps = psO.tile([P, AC], F32, tag="ops")
                            l_ps = psL.tile([P, AC], F32, tag="lps")
                            njb = 4 * qcx + 4
                            for jb in range(njb):
                                d = jb - 4 * qcx  # diag offset if >= 0
                                off = 128 * d if d > 0 else 0
                                s_ps = psS.tile([P, AC], F32, tag="sps")
                                nc.tensor.matmul(
                                    s_ps[:, off:],
                                    Kt[:, h, jb * P : (jb + 1) * P],
                                    Qt[:, h, qcx * AC + off : (qcx + 1) * AC],
                                    start=True,
                                    stop=True,
                                )
                                pt = ptp.tile([P, AC], F16, tag="pt")
                                nc.scalar.activation(
                                    pt[:, off:],
                                    s_ps[:, off:],
                                    mybir.ActivationFunctionType.Exp,
                                    bias=ebias,
                                )
                                if d >= 0:
                                    # zero the upper triangle (kv > q) post-exp
                                    nc.gpsimd.affine_select(
                                        pt[:, off : off + P],
                                        pt[:, off : off + P],
                                        pattern=[[1, P]],
                                        compare_op=mybir.AluOpType.is_ge,
                                        fill=zfill,
                                        base=0,
                                        channel_multiplier=-1,
                                    )
                                nc.tensor.matmul(
                                    o_ps[:, off:],
                                    Vt[:, jb, hs],
                                    pt[:, off:],
                                    start=(jb == 0),
                                    stop=(jb == njb - 1),
                                )
                                nc.tensor.matmul(
                                    l_ps[:, off:],
                                    ones_sb,
                                    pt[:, off:],
                                    start=(jb == 0),
                                    stop=(jb == njb - 1),
                                )
                            rinv = wk2.tile([P, AC], F32, tag="rinv")
                            nc.vector.reciprocal(rinv, l_ps)
                            # yt = (o_ps * YS) * rinv, fp16 (Y scaled by YS
                            # for the fp8 split; undone in the out-proj copy)
                            yt = wk2.tile([P, AC], F16, tag="yt")
                            nc.vector.scalar_tensor_tensor(
                                yt, o_ps, YS, rinv,
                                op0=mybir.AluOpType.mult,
                                op1=mybir.AluOpType.mult,
                            )
                            nc.vector.tensor_copy(Yhi[:, h, win], yt)
                            nc.vector.scalar_tensor_tensor(
                                Ylo[:, h, win], Yhi[:, h, win], -1.0, yt,
                                op0=mybir.AluOpType.mult,
                                op1=mybir.AluOpType.add,
                            )

                # ------------ phase 3: out projection (fp8 DR) -------------
                with (
                    tc.tile_pool(name="outp", bufs=2) as outp,
                    tc.tile_pool(name="ps_p", bufs=4, space="PSUM") as psP,
                ):
                    for tt in range(TT):
                        tw = slice(tt * P, (tt + 1) * P)
                        ob = outp.tile([P, D], F16, tag="ob")
                        for ncx in range(D // QC):
                            nwin = slice(ncx * QC, (ncx + 1) * QC)
                            ps = psP.tile([P, QC], F32, tag="psp")
                            terms = (
                                (Yhi, wp_hi_sb),
                                (Yhi, wp_lo_sb),
                                (Ylo, wp_hi_sb),
                            )
                            for i, (yt8, wt8) in enumerate(terms):
                                for hp in range(NH_CORE // 2):
                                    ks = slice(2 * hp, 2 * hp + 2)
                                    nc.tensor.matmul(
                                        ps,
                                        yt8[:, ks, tw],
                                        wt8[:, ks, nwin],
                                        start=(i == 0 and hp == 0),
                                        stop=(i == 2 and hp == NH_CORE // 2 - 1),
                                        perf_mode=DR,
                                    )
                            if ncx % 2 == 0:
                                nc.scalar.activation(
                                    ob[:, nwin],
                                    ps,
                                    mybir.ActivationFunctionType.Copy,
                                    scale=1.0 / (WS * YS),
                                )
                            else:
                                nc.vector.tensor_scalar_mul(
                                    ob[:, nwin], ps, 1.0 / (WS * YS)
                                )
                        if tt < TT - 1:
                            nc.sync.dma_start(
                                out_t[:, tt, 0:1024], ob[:, 0:1024]
                            )
                            nc.gpsimd.dma_start(
                                out_t[:, tt, 1024:2048], ob[:, 1024:2048]
                            )
                        else:
                            # final tile: small chunks so the drain is short
                            for qi, eng in enumerate(
                                (nc.sync, nc.gpsimd, nc.sync, nc.gpsimd)
                            ):
                                cw = slice(qi * QC, (qi + 1) * QC)
                                eng.dma_start(out_t[:, tt, cw], ob[:, cw])
    return nc


# ---------------------------------------------------------------------------
# legalization: this walrus build supports only ONE sync wait per instruction
# ---------------------------------------------------------------------------
_ENGINE_SEM_PREFIX = {
    "PE": "PE_",
    "DVE": "DVE_",
    "ACT": "ACT_",
    "Pool": "POOL_",
    "SP": "SP_",
}
_wf_counter = [0]


def _legalize(nc, max_waits=1):
    for f in nc.m.functions:
        for bb in f.blocks:
            new_insts = []
            for inst in bb.instructions:
                si = getattr(inst, "sync_info", None)
                eng = getattr(inst, "engine", None)
                if si is None or not si.on_wait or eng is None:
                    new_insts.append(inst)
                    continue
                waits = list(si.on_wait)
                pref = _ENGINE_SEM_PREFIX.get(eng.name)
                if pref is not None:
                    waits = [
                        w
                        for w in waits
                        if not (
                            w.sync_type == "semaphore"
                            and w.ant_name.startswith(pref)
                        )
                    ]
                if len(waits) > max_waits:
                    for w in waits[:-max_waits]:
                        _wf_counter[0] += 1
                        nop = mybir.InstNoOp(
                            name=f"I-waitfix-{_wf_counter[0]}", ins=[], outs=[]
                        )
                        nop.engine = eng
                        nop.sync_info = mybir.SyncInfo(on_wait=[w], on_update=[])
                        new_insts.append(nop)
                    waits = waits[-max_waits:]
                if len(waits) != len(si.on_wait):
                    inst.sync_info = mybir.SyncInfo(
                        on_wait=waits, on_update=list(si.on_update)
                    )
                new_insts.append(inst)
            bb.instructions[:] = new_insts


# ---------------------------------------------------------------------------
# SPMD runner (mirrors concourse.bass2jax.run_bass_via_pjrt, kept resident)
# ---------------------------------------------------------------------------
class _Runner:
    def __init__(self, nc, n_cores=8):
        import jax
        from jax.sharding import Mesh, PartitionSpec
        from jax.experimental.shard_map import shard_map
        from concourse import bass2jax
        from concourse.bass2jax import _bass_exec_p, install_neuronx_cc_hook

        install_neuronx_cc_hook()
        self.jax = jax
        self.nc = nc
        self.n_cores = n_cores
        partition_name = (
            nc.partition_id_tensor.name if nc.partition_id_tensor else None
        )
        in_names, out_names, out_avals, zero_outs = [], [], [], []
        for alloc in nc.m.functions[0].allocations:
            if not isinstance(alloc, mybir.MemoryLocationSet):
                continue
            name = alloc.memorylocations[0].name
            if alloc.kind == "ExternalInput":
                if name != partition_name:
                    in_names.append(name)
            elif alloc.kind == "ExternalOutput":
                shape = tuple(alloc.tensor_shape)
                dtype = mybir.dt.np(alloc.dtype)
                out_names.append(name)
                out_avals.append(jax.core.ShapedArray(shape, dtype))
                zero_outs.append(np.zeros(shape, dtype))
        self.in_names, self.out_names = in_names, out_names
        self.out_avals, self.zero_outs = out_avals, zero_outs
        n_params, n_outs = len(in_names), len(out_names)
        all_in_names = in_names + out_names
        if partition_name is not None:
            all_in_names.append(partition_name)
        donate = tuple(range(n_params, n_params + n_outs))

        def _body(*args):
            operands = list(args)
            if partition_name is not None:
                operands.append(bass2jax.partition_id_tensor())
            return tuple(
                _bass_exec_p.bind(
                    *operands,
                    out_avals=tuple(out_avals),
                    in_names=tuple(all_in_names),
                    out_names=tuple(out_names),
                    lowering_input_output_aliases=(),
                    sim_require_finite=True,
                    sim_require_nnan=True,
                    nc=nc,
                )
            )

        devices = jax.devices()[:n_cores]
        mesh = Mesh(np.asarray(devices), ("core",))
        in_specs = (PartitionSpec("core"),) * (n_params + n_outs)
        out_specs = (PartitionSpec("core"),) * n_outs
        self.fn = jax.jit(
            shard_map(
                _body,
                mesh=mesh,
                in_specs=in_specs,
                out_specs=out_specs,
                check_rep=False,
            ),
            donate_argnums=donate,
            keep_unused=True,
        )

    def run(self, in_maps):
        n = self.n_cores
        concat_in = [
            np.concatenate(
                [np.asarray(in_maps[c][name]) for c in range(n)], axis=0
            )
            for name in self.in_names
        ]
        zeros = [
            np.zeros((n * z.shape[0], *z.shape[1:]), z.dtype)
            for z in self.zero_outs
        ]
        out_arrs = self.fn(*concat_in, *zeros)
        return [
            {
                name: np.asarray(out_arrs[i]).reshape(
                    n, *self.out_avals[i].shape
                )[c]
                for i, name in enumerate(self.out_names)
            }
            for c in range(n)
        ]


_RUNNER = None


def _get_runner():
    global _RUNNER
    if _RUNNER is None:
        nc = _build_nc()
        _legalize(nc)
        _RUNNER = _Runner(nc, 8)
    return _RUNNER


# ---------------------------------------------------------------------------
# public entry point
# ---------------------------------------------------------------------------
def kernel(x, Wqkv, Wproj):
    x = np.asarray(x, dtype=np.float32)
    Wqkv = np.asarray(Wqkv, dtype=np.float32)
    Wproj = np.asarray(Wproj, dtype=np.float32)
    perm = _dim_perm()

    xT8 = [
        _split8(np.ascontiguousarray(x[b].T)) for b in range(B)
    ]  # (hi, lo) per batch
    in_maps = []
    for c in range(8):
        b, g = c // 4, c % 4
        heads = range(NH_CORE * g, NH_CORE * (g + 1))
        qcols = np.concatenate([h * HD + perm for h in heads])
        wq = _split8(Wqkv[:, 0 * D + qcols] * WS)
        wkk = _split8(Wqkv[:, 1 * D + qcols] * WS)
        wv = _split8(
            Wqkv[:, 2 * D + g * HCOLS : 2 * D + (g + 1) * HCOLS] * WS
        )
        wp = _split8(Wproj[g * HCOLS : (g + 1) * HCOLS, :] * WS)
        in_maps.append(
            {
                "xhi": xT8[b][0], "xlo": xT8[b][1],
                "wq_hi": wq[0], "wq_lo": wq[1],
                "wk_hi": wkk[0], "wk_lo": wkk[1],
                "wv_hi": wv[0], "wv_lo": wv[1],
                "wp_hi": wp[0], "wp_lo": wp[1],
            }
        )

    results = _get_runner().run(in_maps)
    out = np.zeros((B, S, D), dtype=np.float32)
    for c in range(8):
        out[c // 4] += results[c]["out"].astype(np.float32)
    return out
